# revision 34
# baseline (speedup 1.0000x reference)
"""DySkillHGNN Trainium2 kernel: 6 timesteps x (GAT_p + GAT_c + SAGE) over 30000 nodes.

Output sparsity: the model only returns rows s (~1015 unique node ids) of each
timestep, so only edges whose destination is in unique(s) matter. Slots
g = t*U + slot(dst) (~6090 of them) are packed into 48 blocks of 128 and
sharded 6-per-core across 8 NeuronCores. Edges are bucketed by slot block,
padded to a uniform tile count. Each core computes out rows for its blocks;
host reassembles and expands to [6, |s|, D].

Per 128-slot block (T tiles of 128 edges per relation):
 - GAT: gather rows [h(128)|1.0|a_s|pad] (256 bf16 = 512B) by src; one merged
   gather of [a_d_p|a_d_c|pad] rows (256B) for both GAT relations' dst ids.
 - SAGE: gather rows of x@W_l (128 bf16) by src; 1/(3*max(cnt,1)) comes from
   the host (pure index data).
 - One-hot scatter matrices are HOST-precomputed from the (index-only) edge
   buckets and DMA'd sequentially on the ACT HWDGE ring -- no per-tile DVE
   work and no per-tile cross-engine semaphores.
 - exp(leaky_relu(a_s+a_d)) weights fold into the h rows with one big
   elementwise multiply per relation; the constant-1.0 table column then
   makes PSUM column 128 of the scatter matmul the softmax denominator.
 - Scatter matmul lhsT=onehot, rhs=h gives num[dst, dim] directly; epilogue
   is transpose-free: out = num_p*r_p + num_c*r_c + num_s*r_s + xr3b.
"""

import numpy as np
import ml_dtypes

import concourse.bass as bass
import concourse.tile as tile
from concourse import bacc, mybir
import concourse.bass_utils as bass_utils
from concourse.masks import make_identity

P = 128
DIM = 128
CORES = 8
HWB = 256                         # GAT H table width (bf16): [h(128)|1|a_s|pad]
AWB = 128                         # A_D table width (bf16): [a_d_p|a_d_c|pad]
NEG = 0.2
F32 = mybir.dt.float32
BF16 = mybir.dt.bfloat16
I16 = mybir.dt.int16
NPBF = ml_dtypes.bfloat16

N_NODES = 30000
N_T = 6
NPAD = ((N_NODES + 1 + P - 1) // P) * P   # 30080 table rows
PADROW = N_NODES                  # index of the all-zero row
XRW = 132                         # xr3b row: [x@(W_r/3)+bias (128) | r_s | pad]

_CB = [6]                         # blocks per core (set by _run before build)
_MODE = ["full"]                  # build mode ("gather" ablation)
_SP = [False]                     # dma_gather single_packet
_QS = ["pair"]                    # gather queue strategy: rot | same | pair
_GSORT = [True]                   # sort GAT edges by src (h locality)
_OH8 = [False]                    # one-hot tiles in fp8 (half the DMA bytes)


# ---------------------------------------------------------------- host prep

def _prep_rel(edge_index, r, selfloop, uniq, slot_of, U, NBLK, T=None,
              sort_src=False):
    """Bucket one relation's kept edges (all t) by slot block.

    Returns (src_pad, dst_pad, dloc_pad, cnt, T); *_pad have shape
    [NBLK, T*P] with PADROW / -1 padding; cnt is real edges per slot.
    sort_src orders edges within a block by source id (better HBM locality
    for the h-gather; any within-block order is valid).
    """
    gs, srcs = [], []
    for t in range(N_T):
        src = np.asarray(edge_index[t, r, 0])
        dst = np.asarray(edge_index[t, r, 1])
        sl = slot_of[dst]
        keep = sl >= 0
        g = t * U + sl[keep]
        sk = src[keep]
        if selfloop:
            g = np.concatenate([g, t * U + np.arange(U, dtype=np.int64)])
            sk = np.concatenate([sk, uniq])
        gs.append(g)
        srcs.append(sk)
    g = np.concatenate(gs)
    src = np.concatenate(srcs)
    if sort_src:
        order = np.argsort((g // P) * (N_NODES + 1) + src, kind="stable")
    else:
        order = np.argsort(g, kind="stable")
    gs_ = g[order]
    src_ = src[order]
    blk = gs_ // P
    cnt = np.bincount(blk, minlength=NBLK)
    if T is None:
        T = int(-(-cnt.max() // P))
    EB = T * P
    assert cnt.max() <= EB, (cnt.max(), EB)
    starts = np.concatenate([[0], np.cumsum(cnt)[:-1]])
    pos = np.arange(len(gs_)) - np.repeat(starts, cnt)
    src_pad = np.full((NBLK, EB), PADROW, np.int64)
    dst_pad = np.full((NBLK, EB), PADROW, np.int64)
    dloc_pad = np.full((NBLK, EB), -1, np.int64)
    src_pad[blk, pos] = src_
    dst_pad[blk, pos] = uniq[gs_ % U]
    dloc_pad[blk, pos] = gs_ % P
    slot_cnt = np.bincount(gs_, minlength=NBLK * P)
    return src_pad, dst_pad, dloc_pad, slot_cnt, T


def _count_tiles(edge_index, r, selfloop, slot_of, U, NBLK):
    gs = []
    for t in range(N_T):
        dst = np.asarray(edge_index[t, r, 1])
        sl = slot_of[dst]
        g = t * U + sl[sl >= 0]
        gs.append(g)
        if selfloop:
            gs.append(t * U + np.arange(U, dtype=np.int64))
    cnt = np.bincount(np.concatenate(gs) // P, minlength=NBLK)
    return max(1, int(-(-cnt.max() // P)))


def _idx_image(a):
    """[..., E_b] int -> DMA-ready int16 image [..., 128, E_b//16]."""
    S = a.shape[-1] // 16
    w = a.reshape(*a.shape[:-1], S, 16)                     # [..., S, 16]
    w = np.swapaxes(w, -1, -2)                              # [..., 16, S]
    w = np.broadcast_to(w[..., None, :, :], (*a.shape[:-1], 8, 16, S))
    return np.ascontiguousarray(w.reshape(*a.shape[:-1], 128, S)).astype(np.int16)


def _onehot_image(dlocs, NBLK, dt=NPBF):
    """list of [NBLK, T*P] dloc arrays -> [P, NBLK, TT, P] bf16 one-hots
    where TT = sum of T over relations; partition axis = edge-in-tile."""
    pieces = []
    for dl in dlocs:
        T = dl.shape[-1] // P
        pieces.append(dl.reshape(NBLK, T, P))
    dl_all = np.concatenate(pieces, axis=1)                 # [NBLK, TT, P]
    TT = dl_all.shape[1]
    oh = np.zeros((NBLK, TT, P, P), dt)
    b, k, e = np.nonzero(dl_all >= 0)
    oh[b, k, e, dl_all[b, k, e]] = 1.0
    return np.ascontiguousarray(oh.transpose(2, 0, 1, 3))   # [P, NBLK, TT, P]


def _onehotT_image(dlocs, NBLK, dt=NPBF):
    """Transposed one-hots [dst, edge] for the a_d lookup matmuls:
    -> [P(dst), NBLK, TT, P(edge)] bf16."""
    pieces = []
    for dl in dlocs:
        T = dl.shape[-1] // P
        pieces.append(dl.reshape(NBLK, T, P))
    dl_all = np.concatenate(pieces, axis=1)                 # [NBLK, TT, P]
    TT = dl_all.shape[1]
    oh = np.zeros((NBLK, TT, P, P), dt)                     # [b, k, dst, e]
    b, k, e = np.nonzero(dl_all >= 0)
    oh[b, k, dl_all[b, k, e], e] = 1.0
    return np.ascontiguousarray(oh.transpose(2, 0, 1, 3))   # [P, NBLK, TT, P]


# ---------------------------------------------------------------- device program

def build_program(T_G, T_S, repeats=1, mode=None):
    """Build the SPMD Bass program. T_G / T_S: tiles per GAT / SAGE block."""
    if mode is None:
        mode = _MODE[0]
    S_G, S_S = T_G * 8, T_S * 8          # idx image cols (= T*128/16)
    CB = _CB[0]                          # blocks per core
    SW = 2 * S_G + S_S + 8               # merged idx image width [sp|sc|ss|dst]
    TT = 2 * T_G + T_S                   # one-hot tiles per block
    TG2 = 2 * T_G                        # transposed one-hot tiles per block

    nc = bacc.Bacc("TRN2", target_bir_lowering=False, debug=False,
                   num_devices=CORES, num_swdge_queues=4)

    def din(name, shape, dt=F32):
        return nc.dram_tensor(name, shape, dt, kind="ExternalInput")

    x_t = din("x_t", [P, NPAD])                  # x transposed (host)
    w_p = din("w_p", [DIM, DIM])
    w_c = din("w_c", [DIM, DIM])
    w_l = din("w_l", [DIM, DIM])
    att = din("att", [DIM, 4])                   # [src_p | dst_p | src_c | dst_c]
    # prefetched per-rep metadata, already transposed to [128, CB*W] on host
    sidx = din("sidx", [P, CB * SW], I16)
    xr3b = din("xr3b", [P, CB * XRW])
    OHDT = mybir.dt.float8e4 if _OH8[0] else BF16
    oh_d = din("oh", [P, CB * TT * P], OHDT)     # host one-hot tiles [e, dst]
    oht_d = din("oht", [P, CB * TG2 * P], OHDT)  # transposed one-hots [dst, e]

    out_d = nc.dram_tensor("out", [CB * P, DIM], F32, kind="ExternalOutput")

    ht_p = nc.dram_tensor("ht_p", [NPAD, HWB], BF16, kind="Internal")
    ht_c = nc.dram_tensor("ht_c", [NPAD, HWB], BF16, kind="Internal")
    ad_pc = nc.dram_tensor("ad_pc", [NPAD, AWB], BF16, kind="Internal")
    hl_t = nc.dram_tensor("hl_t", [NPAD, DIM], BF16, kind="Internal")

    with tile.TileContext(nc) as tc:
        with tc.tile_pool(name="const", bufs=1) as cpool:
            ident = cpool.tile([P, P], F32)
            make_identity(nc, ident[:])

            # ---------------- prologue: build H / A_D / HL tables ----------------
            with tc.tile_pool(name="prol", bufs=3) as pp, \
                 tc.tile_pool(name="prolp", bufs=2, space="PSUM") as ppp, \
                 tc.tile_pool(name="xtp", bufs=2) as xtp:
                wp_t = pp.tile([DIM, DIM], F32, tag="wp")
                nc.sync.dma_start(wp_t[:], w_p[:])
                wc_t = pp.tile([DIM, DIM], F32, tag="wc")
                nc.sync.dma_start(wc_t[:], w_c[:])
                wl_t = pp.tile([DIM, DIM], F32, tag="wl")
                nc.sync.dma_start(wl_t[:], w_l[:])
                att_t = pp.tile([DIM, 4], F32, tag="att")
                nc.sync.dma_start(att_t[:], att[:])
                wpT_ps = ppp.tile([DIM, DIM], F32, tag="wT")
                nc.tensor.transpose(wpT_ps[:], wp_t[:], ident[:])
                wpT = pp.tile([DIM, DIM], F32, tag="wpT")
                nc.scalar.copy(wpT[:], wpT_ps[:])
                wcT_ps = ppp.tile([DIM, DIM], F32, tag="wT")
                nc.tensor.transpose(wcT_ps[:], wc_t[:], ident[:])
                wcT = pp.tile([DIM, DIM], F32, tag="wcT")
                nc.scalar.copy(wcT[:], wcT_ps[:])
                v_ps = ppp.tile([DIM, 4], F32, tag="v")
                nc.tensor.matmul(v_ps[:, 0:2], lhsT=wpT[:], rhs=att_t[:, 0:2],
                                 start=True, stop=True)
                nc.tensor.matmul(v_ps[:, 2:4], lhsT=wcT[:], rhs=att_t[:, 2:4],
                                 start=True, stop=True)
                v_t = pp.tile([DIM, 4], F32, tag="vt")
                nc.scalar.copy(v_t[:], v_ps[:])

                NQ = 5
                QCH = (NPAD // P + NQ - 1) // NQ
                for q in range(NQ):
                  c_lo = q * QCH
                  c_hi = min((q + 1) * QCH, NPAD // P)
                  xts = xtp.tile([P, QCH * P], F32, tag="xts")
                  nc.sync.dma_start(xts[:, 0:(c_hi - c_lo) * P],
                                    x_t[:, c_lo * P:c_hi * P])
                  for c in range(c_lo, c_hi):
                    sl = slice(c * P, (c + 1) * P)
                    xt_c = xts[:, (c - c_lo) * P:(c - c_lo + 1) * P]
                    a_ps = ppp.tile([P, 4], F32, tag="a")
                    nc.tensor.matmul(a_ps[:, 0:2], lhsT=xt_c, rhs=v_t[:, 0:2],
                                     start=True, stop=True)
                    nc.tensor.matmul(a_ps[:, 2:4], lhsT=xt_c, rhs=v_t[:, 2:4],
                                     start=True, stop=True)
                    for (wt, htab, a_col) in ((wp_t, ht_p, 0), (wc_t, ht_c, 2)):
                        h_ps = ppp.tile([P, DIM], F32, tag="h")
                        nc.tensor.matmul(h_ps[:], lhsT=xt_c, rhs=wt[:],
                                         start=True, stop=True)
                        htile = pp.tile([P, HWB], BF16, tag="htile")
                        nc.scalar.copy(htile[:, 0:DIM], h_ps[:])
                        nc.vector.memset(htile[:, DIM:DIM + 1], 1.0)
                        nc.scalar.copy(htile[:, DIM + 1:DIM + 2],
                                       a_ps[:, a_col:a_col + 1])
                        nc.vector.memset(htile[:, DIM + 2:], 0.0)
                        nc.sync.dma_start(htab[sl, :], htile[:])
                    atile = pp.tile([P, AWB], BF16, tag="atile")
                    nc.scalar.copy(atile[:, 0:1], a_ps[:, 1:2])
                    nc.scalar.copy(atile[:, 1:2], a_ps[:, 3:4])
                    nc.vector.memset(atile[:, 2:], 0.0)
                    nc.sync.dma_start(ad_pc[sl, :], atile[:])
                    hl_ps = ppp.tile([P, DIM], F32, tag="h")
                    nc.tensor.matmul(hl_ps[:], lhsT=xt_c, rhs=wl_t[:],
                                     start=True, stop=True)
                    hltile = pp.tile([P, DIM], BF16, tag="hltile")
                    nc.scalar.copy(hltile[:], hl_ps[:])
                    nc.sync.dma_start(hl_t[sl, :], hltile[:])

            # ---------------- edge phase ----------------
            with tc.tile_pool(name="meta", bufs=2) as mp, \
                 tc.tile_pool(name="gath", bufs=3) as gp, \
                 tc.tile_pool(name="ohp", bufs=3) as ohp, \
                 tc.tile_pool(name="wrk", bufs=3) as wp_pool, \
                 tc.tile_pool(name="evac", bufs=3) as ep, \
                 tc.tile_pool(name="eps", bufs=2, space="PSUM") as eps:

                def epilogue(blk, num_p, num_c, num_s, xr_t):
                    rows = bass.ds(blk * P, P)
                    r_p = ep.tile([P, 1], F32, tag="r_p")
                    nc.vector.tensor_scalar(out=r_p[:], in0=num_p[:, DIM:DIM + 1],
                                            scalar1=3.0, scalar2=None,
                                            op0=mybir.AluOpType.mult)
                    nc.vector.reciprocal(r_p[:], r_p[:])
                    r_c = ep.tile([P, 1], F32, tag="r_c")
                    nc.vector.tensor_scalar(out=r_c[:], in0=num_c[:, DIM:DIM + 1],
                                            scalar1=3.0, scalar2=None,
                                            op0=mybir.AluOpType.mult)
                    nc.vector.reciprocal(r_c[:], r_c[:])
                    acc = ep.tile([P, DIM], F32, tag="acc")
                    nc.scalar.activation(acc[:], num_p[:, 0:DIM],
                                         mybir.ActivationFunctionType.Copy,
                                         scale=r_p[:])
                    acc2 = ep.tile([P, DIM], F32, tag="acc2")
                    nc.vector.scalar_tensor_tensor(
                        out=acc2[:], in0=num_c[:, 0:DIM], scalar=r_c[:],
                        in1=acc[:], op0=mybir.AluOpType.mult,
                        op1=mybir.AluOpType.add)
                    acc3 = ep.tile([P, DIM], F32, tag="acc3")
                    nc.vector.scalar_tensor_tensor(
                        out=acc3[:], in0=num_s[:],
                        scalar=xr_t[:, DIM:DIM + 1],
                        in1=acc2[:], op0=mybir.AluOpType.mult,
                        op1=mybir.AluOpType.add)
                    out_t = ep.tile([P, DIM], F32, tag="out_t")
                    nc.vector.tensor_add(out_t[:], acc3[:], xr_t[:, 0:DIM])
                    nc.sync.dma_start(out_d[rows, :], out_t[:])

                for _rep in range(repeats):
                  si_all = mp.tile([P, CB * SW], I16, tag="si")
                  nc.sync.dma_start(si_all[:], sidx[:])
                  xr_all = mp.tile([P, CB * XRW], F32, tag="xr")
                  nc.sync.dma_start(xr_all[:], xr3b[:])
                  pending = None
                  for blk in range(CB):
                      rows = bass.ds(blk * P, P)
                      si_t = si_all[:, blk * SW:(blk + 1) * SW]
                      xr_t = xr_all[:, blk * XRW:(blk + 1) * XRW]

                      # host one-hots for this block, on the ACT HWDGE ring
                      oh_t = ohp.tile([P, TT, P], OHDT, tag="oh")
                      nc.scalar.dma_start(
                          oh_t[:], oh_d[:, blk * TT * P:(blk + 1) * TT * P]
                          .rearrange("p (t d) -> p t d", t=TT))
                      oht_t = ohp.tile([P, TG2, P], OHDT, tag="oht")
                      nc.scalar.dma_start(
                          oht_t[:], oht_d[:, blk * TG2 * P:(blk + 1) * TG2 * P]
                          .rearrange("p (t d) -> p t d", t=TG2))

                      num_p = eps.tile([P, DIM + 2], F32, tag="num_p")
                      num_c = eps.tile([P, DIM + 2], F32, tag="num_c")
                      num_s = eps.tile([P, DIM], F32, tag="num_s")
                      adE = eps.tile([P, TG2], F32, tag="adE")

                      # ---- gathers
                      if _QS[0] == "same":
                          qn = [blk % 4] * 4
                      elif _QS[0] == "pair":
                          qn = [(2 * blk) % 4, (2 * blk) % 4,
                                (2 * blk + 1) % 4, (2 * blk + 1) % 4]
                      else:
                          qn = [(blk + i) % 4 for i in range(4)]
                      spk = _SP[0]
                      hg_p = gp.tile([P, T_G, HWB], BF16, tag="hgp")
                      nc.gpsimd.dma_gather(hg_p[:], ht_p[:], si_t[:, 0:S_G],
                                           T_G * P, T_G * P, HWB,
                                           single_packet=spk,
                                           queue_num=qn[0])
                      hg_c = gp.tile([P, T_G, HWB], BF16, tag="hgc")
                      nc.gpsimd.dma_gather(hg_c[:], ht_c[:], si_t[:, S_G:2 * S_G],
                                           T_G * P, T_G * P, HWB,
                                           single_packet=spk,
                                           queue_num=qn[1])
                      hlg = gp.tile([P, T_S, DIM], BF16, tag="hlg")
                      nc.gpsimd.dma_gather(hlg[:], hl_t[:],
                                           si_t[:, 2 * S_G:2 * S_G + S_S],
                                           T_S * P, T_S * P, DIM,
                                           single_packet=spk,
                                           queue_num=qn[2])
                      # a_d for the block's 128 dst slots (one row each)
                      adc = gp.tile([P, 1, AWB], BF16, tag="adc")
                      nc.gpsimd.dma_gather(adc[:], ad_pc[:],
                                           si_t[:, 2 * S_G + S_S:],
                                           P, P, AWB,
                                           single_packet=spk,
                                           queue_num=qn[3])

                      if mode == "gather":
                          zo = ep.tile([P, DIM], F32, tag="out_t")
                          nc.vector.memset(zo[:], 0.0)
                          nc.sync.dma_start(out_d[rows, :], zo[:])
                          continue

                      # ---- per-edge a_d via PE: adE[e,k] = ohT_k^T @ ad_col
                      for k in range(TG2):
                          nc.tensor.matmul(adE[:, k:k + 1],
                                           lhsT=oht_t[:, k, :],
                                           rhs=adc[:, 0, (k // T_G):(k // T_G) + 1],
                                           start=True, stop=True)
                      # ---- GAT relations: w = exp(leaky(a_s+a_d)), fold into h
                      for (ri, hg) in ((0, hg_p), (1, hg_c)):
                          s_t = wp_pool.tile([P, T_G], F32, tag=f"s{ri}")
                          nc.vector.tensor_tensor(
                              out=s_t[:], in0=hg[:, :, DIM + 1],
                              in1=adE[:, ri * T_G:(ri + 1) * T_G],
                              op=mybir.AluOpType.add)
                          lr_t = wp_pool.tile([P, T_G], F32, tag=f"lr{ri}")
                          nc.scalar.activation(
                              lr_t[:], s_t[:],
                              mybir.ActivationFunctionType.Prelu, alpha=NEG)
                          w_t = wp_pool.tile([P, T_G], F32, tag=f"w{ri}")
                          nc.scalar.activation(
                              w_t[:], lr_t[:],
                              mybir.ActivationFunctionType.Exp)
                          # hw = h * w in place (cols 0..129; col 128 -> w),
                          # split in halves so scatter matmuls start earlier
                          TH = T_G // 2
                          nc.vector.tensor_tensor(
                              out=hg[:, 0:TH, 0:DIM + 2],
                              in0=hg[:, 0:TH, 0:DIM + 2],
                              in1=w_t[:, 0:TH].unsqueeze(2)
                                  .to_broadcast([P, TH, DIM + 2]),
                              op=mybir.AluOpType.mult)
                          nc.vector.tensor_tensor(
                              out=hg[:, TH:T_G, 0:DIM + 2],
                              in0=hg[:, TH:T_G, 0:DIM + 2],
                              in1=w_t[:, TH:T_G].unsqueeze(2)
                                  .to_broadcast([P, T_G - TH, DIM + 2]),
                              op=mybir.AluOpType.mult)
                      # ---- scatter matmuls (lhsT = host one-hot)
                      for (ri, hg, num_ps) in ((0, hg_p, num_p), (1, hg_c, num_c)):
                          for k in range(T_G):
                              nc.tensor.matmul(num_ps[:],
                                               lhsT=oh_t[:, ri * T_G + k, :],
                                               rhs=hg[:, k, 0:DIM + 2],
                                               start=(k == 0),
                                               stop=(k == T_G - 1))
                      for k in range(T_S):
                          nc.tensor.matmul(num_s[:],
                                           lhsT=oh_t[:, 2 * T_G + k, :],
                                           rhs=hlg[:, k, :],
                                           start=(k == 0),
                                           stop=(k == T_S - 1))

                      # epilogue deferred one block so DVE never stalls on PE
                      if pending is not None:
                          epilogue(*pending)
                      pending = (blk, num_p, num_c, num_s, xr_t)
                  if pending is not None:
                      epilogue(*pending)

    nc.compile()
    return nc


# ---------------------------------------------------------------- entry

def _run(inputs, trace=False):
    s = np.asarray(inputs["s"])
    edge_index = np.asarray(inputs["edge_index"])
    x = np.asarray(inputs["embed_weight"], dtype=np.float32)
    W_p = np.asarray(inputs["W_p"], np.float32)
    W_c = np.asarray(inputs["W_c"], np.float32)
    W_l = np.asarray(inputs["W_l"], np.float32)
    W_r = np.asarray(inputs["W_r"], np.float32)
    att = np.stack([np.asarray(inputs["att_src_p"], np.float32),
                    np.asarray(inputs["att_dst_p"], np.float32),
                    np.asarray(inputs["att_src_c"], np.float32),
                    np.asarray(inputs["att_dst_c"], np.float32)], axis=1)
    bias = ((np.asarray(inputs["b_p"], np.float32)
             + np.asarray(inputs["b_c"], np.float32)
             + np.asarray(inputs["b_l"], np.float32)
             + np.asarray(inputs["b_r"], np.float32)) / 3.0)[None, :]

    uniq, inv = np.unique(s, return_inverse=True)
    U = len(uniq)
    G = N_T * U                               # real slots
    NBLK = -(-G // P)                         # total 128-slot blocks
    CB = -(-NBLK // CORES)                    # blocks per core
    NBLK = CB * CORES                         # padded to core multiple
    _CB[0] = CB
    slot_of = np.full(N_NODES, -1, np.int64)
    slot_of[uniq] = np.arange(U)

    T_G = max(_count_tiles(edge_index, r, True, slot_of, U, NBLK)
              for r in (0, 1))
    T_S = _count_tiles(edge_index, 2, False, slot_of, U, NBLK)

    rel_data = {}
    for name, r, loop, T in (("p", 0, True, T_G), ("c", 1, True, T_G),
                             ("s", 2, False, T_S)):
        sp, dp, dl, cnt, _ = _prep_rel(edge_index, r, loop, uniq, slot_of,
                                       U, NBLK, T,
                                       sort_src=(name == "s" or _GSORT[0]))
        rel_data[name] = (sp, dp, dl, cnt, T)

    x_pad = np.zeros((NPAD, DIM), np.float32)
    x_pad[:N_NODES] = x
    x_t = np.ascontiguousarray(x_pad.T)
    xr3 = x_pad @ (W_r / 3.0) + bias

    # per-slot dst node ids (PADROW for padding slots)
    slots = np.arange(NBLK * P)
    valid = slots < G
    node_of_slot = np.where(valid, uniq[np.minimum(slots % U, U - 1)], PADROW)
    node_of_slot[~valid] = PADROW
    xr3b = np.zeros((NBLK * P, XRW), np.float32)
    xr3b[:, 0:DIM] = xr3[node_of_slot]
    cnt_s = rel_data["s"][3].astype(np.float32)
    xr3b[:, DIM] = 1.0 / (3.0 * np.maximum(cnt_s, 1.0))

    nc = build_program(T_G, T_S)

    S_G, S_S = T_G * 8, T_S * 8
    sidx_img = np.concatenate(
        [_idx_image(rel_data["p"][0]),          # sp  [NBLK, 128, S_G]
         _idx_image(rel_data["c"][0]),          # sc
         _idx_image(rel_data["s"][0]),          # ss  [NBLK, 128, S_S]
         _idx_image(node_of_slot.reshape(NBLK, P))],   # dst ids [NBLK, 128, 8]
        axis=-1)                                # [NBLK, 128, SW]
    ohdt = ml_dtypes.float8_e4m3 if _OH8[0] else NPBF
    oh_img = _onehot_image(
        [rel_data["p"][2], rel_data["c"][2], rel_data["s"][2]],
        NBLK, ohdt)                             # [P, NBLK, TT, P]
    oht_img = _onehotT_image(
        [rel_data["p"][2], rel_data["c"][2]],
        NBLK, ohdt)                             # [P, NBLK, 2*T_G, P]

    in_maps = []
    for core in range(CORES):
        bsl = slice(core * CB, (core + 1) * CB)
        m = {
            "x_t": x_t, "w_p": W_p, "w_c": W_c, "w_l": W_l, "att": att,
            "xr3b": np.ascontiguousarray(
                xr3b[core * CB * P:(core + 1) * CB * P]
                .reshape(CB, P, XRW).transpose(1, 0, 2).reshape(P, CB * XRW)),
            "sidx": np.ascontiguousarray(
                sidx_img[bsl].transpose(1, 0, 2).reshape(P, -1)),
            "oh": np.ascontiguousarray(
                oh_img[:, bsl].reshape(P, -1)),
            "oht": np.ascontiguousarray(
                oht_img[:, bsl].reshape(P, -1)),
        }
        in_maps.append(m)

    res = bass_utils.run_bass_kernel_spmd(nc, in_maps,
                                          core_ids=list(range(CORES)),
                                          trace=trace)
    full = np.concatenate([res.results[c]["out"] for c in range(CORES)],
                          axis=0)                           # [NBLK*P, DIM]
    sel = (np.arange(N_T)[:, None] * U + inv[None, :])      # [N_T, |s|]
    return full[sel], res


def kernel(**inputs):
    """Full (unsharded) inputs -> full output [N_T, len(s), DIM] float32."""
    out, _ = _run(inputs)
    return out
